# revision 4
# baseline (speedup 1.0000x reference)
"""Causal attention kernel for Trainium2 (Bass/Tile), data-parallel over 8 NeuronCores.

Problem (hardcoded): B=32, LQ=LK=1024, D=512, fp32.
  scores = (Q @ K^T)/sqrt(D), causal mask, softmax over keys, out = weights @ V.
  No padding; attn_mask == causal tril (baked in).

v3 design (fp8 DoubleRow + mixed-precision edges):
  - Bulk math in fp8e4 with DoubleRow matmuls (two stacked 128-contractions
    per instruction at 0.5 cycles/row).
  - S^T blocks [128k x 512q] -> PSUM; exp on ACT (scale=1/sqrt(D), bias=-1.5,
    cancels in softmax ratio; keeps exp() in fp8 range) -> fp8 P^T in SBUF.
  - Causal triangles zeroed post-exp by GPSIMD affine_select (Pool engine).
  - Rows q<128 (short, concentrated softmax) use an fp16 path end-to-end:
    fp16 Q/K (shipped separately; fp8 Q drops its dead first 128 columns),
    fp16 scores, fp16 P^T, fp16 V block 0. Rows 128..255 additionally use
    fp16 V blocks 0-1 (weights stay fp8).
  - Row sums via matmul against a (3/64)-valued ones vector (fp8-exact), so
    the uint8 output scale folds into the softmax reciprocal.
  - Output u8 = round(o_norm*(64/3) + 127); host: (u8-127)*3/64. (The DVE
    u8 conversion rounds to nearest; the integer offset avoids fractional-
    immediate lowering surprises.)
    |o_norm| <= max|fp8(V)| ~ 5.8 < 127*3/64 = 5.95.
  - All DMA on the SP ring; loads prefetched 2 batches ahead so store waits
    never head-block a needed load. A small warm-up matmul train ramps the
    PE p-state before real work arrives.
"""

import os
import numpy as np
from contextlib import ExitStack

import ml_dtypes
import concourse.bacc as bacc
import concourse.tile as tile
from concourse import mybir
from concourse.bass_utils import run_bass_kernel_spmd

B, LQ, LK, D = 32, 1024, 1024, 512
N_CORES = 8
BPC = B // N_CORES          # batches per core
P = 128                     # partition dim
NJ = LK // P                # 8 k-blocks
QC = 512                    # q-chunk width
NQC = LQ // QC              # 2 q-chunks
ND2 = D // (2 * P)          # 2 d-chunk pairs
Q8C = LQ - P                # fp8 Q^T columns (128..1024)
SCALE = float(1.0 / np.sqrt(D))
EXP_BIAS = -1.5             # exp(s*SCALE + EXP_BIAS); cancels in softmax ratio
ONES_VAL = 3.0 / 64.0       # fp8-exact; folds output quant scale into recip
OUT_SCALE = 3.0 / 64.0      # host: o = (u8 - 127) * OUT_SCALE

DR = mybir.MatmulPerfMode.DoubleRow

DBG_NB = BPC
WARMUP = 80
FINE = True

_NC_CACHE = {}


def _build(repeat: int = 1):
    f32 = mybir.dt.float32
    f16 = mybir.dt.float16
    f8 = mybir.dt.float8e4
    u8 = mybir.dt.uint8

    nc = bacc.Bacc("TRN2", target_bir_lowering=False, debug=False)
    ktd = nc.declare_dram_parameter("kt", [BPC, P, 4, LK], f8, isOutput=False)
    qtd = nc.declare_dram_parameter("qt", [BPC, P, 4, Q8C], f8, isOutput=False)
    vd = nc.declare_dram_parameter("v", [BPC, P, NJ, D], f8, isOutput=False)
    kt0d = nc.declare_dram_parameter("kt0", [BPC, P, 4, P], f16, isOutput=False)
    qt0d = nc.declare_dram_parameter("qt0", [BPC, P, 4, P], f16, isOutput=False)
    v01d = nc.declare_dram_parameter("v01", [BPC, P, 2, D], f16, isOutput=False)
    outd = nc.declare_dram_parameter("out", [BPC, NQC, P, 4, D], u8, isOutput=True)

    with tile.TileContext(nc) as tc, ExitStack() as ctx:
        const = ctx.enter_context(tc.tile_pool(name="const", bufs=1))
        inp = ctx.enter_context(
            tc.tile_pool(name="inp", bufs=3))
        ptp = ctx.enter_context(
            tc.tile_pool(name="ptp", bufs=3))
        pt0p = ctx.enter_context(tc.tile_pool(name="pt0p", bufs=3))
        osb = ctx.enter_context(tc.tile_pool(name="osb", bufs=4))
        sml = ctx.enter_context(tc.tile_pool(name="sml", bufs=4))
        stp_n = 3
        pvp_n = 4
        stp = ctx.enter_context(tc.tile_pool(name="stp", bufs=stp_n, space="PSUM"))
        pvp = ctx.enter_context(tc.tile_pool(name="pvp", bufs=pvp_n, space="PSUM"))
        smp = ctx.enter_context(tc.tile_pool(name="smp", bufs=1, space="PSUM"))

        # ---- constants ----
        ones8 = const.tile([P, 2, 2], f8)     # [., pair, n] for DoubleRow sums
        nc.gpsimd.memset(ones8[:], ONES_VAL)
        ones16 = const.tile([P, 2], f16)
        nc.gpsimd.memset(ones16[:], ONES_VAL)
        bias_t = const.tile([P, 1], f32)
        nc.gpsimd.memset(bias_t[:], EXP_BIAS)
        wrm = const.tile([P, 2, P], f8)
        nc.gpsimd.memset(wrm[:], 0.25)
        scratch = const.tile([P, 1], f32)

        # preload the Exp activation table while the first loads are in
        # flight (otherwise its 1283ns lands on the first real exp)
        nc.scalar.activation(scratch[:], bias_t[:],
                             mybir.ActivationFunctionType.Exp)

        # warm-up train: keeps the PE continuously busy from ~t0 so the
        # p-state ramp completes before (or during) the first real S^T
        if WARMUP:
            wps = stp.tile([P, QC], f32, tag="st")
            for i in range(WARMUP):
                nc.tensor.matmul(wps[:, 0:64], wrm[:], wrm[:, :, 0:64],
                                 start=(i == 0), stop=(i == WARMUP - 1),
                                 perf_mode=DR, skip_group_check=True)

        def emit_loads(b, fine=False):
            kt_t = inp.tile([P, 4, LK], f8, tag="kt")
            qt_t = inp.tile([P, 4, Q8C], f8, tag="qt")
            v_t = inp.tile([P, NJ, D], f8, tag="v")
            kt0_t = inp.tile([P, 4, P], f16, tag="kt0")
            qt0_t = inp.tile([P, 4, P], f16, tag="qt0")
            v01_t = inp.tile([P, 2, D], f16, tag="v01")
            if fine:
                # first batch: K column-split (512B runs, penalty-free) +
                # whole Q early, so qc0's S^T unblocks after ~1.4us of pipe;
                # V loads are deferred by the caller via the returned thunk
                nc.sync.dma_start(out=kt_t[:, :, 0:QC],
                                  in_=ktd.ap()[b][:, :, 0:QC])
                nc.sync.dma_start(out=qt_t[:], in_=qtd.ap()[b])
                nc.sync.dma_start(out=kt0_t[:], in_=kt0d.ap()[b])
                nc.sync.dma_start(out=qt0_t[:], in_=qt0d.ap()[b])
                nc.sync.dma_start(out=kt_t[:, :, QC:LK],
                                  in_=ktd.ap()[b][:, :, QC:LK])
                nc.sync.dma_start(out=v01_t[:], in_=v01d.ap()[b])
                nc.sync.dma_start(out=v_t[:], in_=vd.ap()[b])
            else:
                nc.sync.dma_start(out=kt_t[:, 0:2], in_=ktd.ap()[b][:, 0:2])
                nc.sync.dma_start(out=qt_t[:, 0:2], in_=qtd.ap()[b][:, 0:2])
                nc.sync.dma_start(out=kt_t[:, 2:4], in_=ktd.ap()[b][:, 2:4])
                nc.sync.dma_start(out=qt_t[:, 2:4], in_=qtd.ap()[b][:, 2:4])
                nc.sync.dma_start(out=kt0_t[:], in_=kt0d.ap()[b])
                nc.sync.dma_start(out=qt0_t[:], in_=qt0d.ap()[b])
                nc.sync.dma_start(out=v01_t[:], in_=v01d.ap()[b])
                nc.sync.dma_start(out=v_t[:], in_=vd.ap()[b])
            return kt_t, qt_t, v_t, kt0_t, qt0_t, v01_t

        def q8cols(lo_global):
            """fp8 Q^T tile column index for global q column (>=128)."""
            return lo_global - P

        def stage_st_units(b, qc, tiles, batch_state):
            """Per-block emission thunks for S^T matmuls + exp (+ hp path).
            Returns (pt, [thunk...]); thunks are interleaved with the pending
            PV stage's thunks so the in-order PE never head-of-line blocks on
            a full stp pool while independent PV work is available."""
            kt_t, qt_t, v_t, kt0_t, qt0_t, v01_t = tiles
            pt = ptp.tile([P, NJ, QC], f8, tag="pt")
            units = []

            def hp_unit():
                # high-precision first block: fp16 scores for rows<128
                hp_full = stp.tile([P, QC], f32, tag="st")
                hp = hp_full[:, 0:P]
                for c in range(4):
                    nc.tensor.matmul(hp[:], kt0_t[:, c, :], qt0_t[:, c, :],
                                     start=(c == 0), stop=(c == 3))
                pt00 = pt0p.tile([P, P], f16, tag="pt00")
                nc.scalar.activation(pt00[:], hp[:],
                                     mybir.ActivationFunctionType.Exp,
                                     scale=SCALE, bias=bias_t[:])
                nc.gpsimd.affine_select(
                    out=pt00[:], in_=pt00[:],
                    compare_op=mybir.AluOpType.is_ge,
                    fill=0.0, base=0, pattern=[[1, P]], channel_multiplier=-1)
                batch_state["pt00"] = pt00

            def st_unit(j):
                r = j - 4 * qc
                lo = P * r if r > 0 else 0
                if qc == 0 and j == 0:
                    lo = P  # rows<128 handled by the fp16 path
                st = stp.tile([P, QC], f32, tag="st")
                for c2 in range(ND2):
                    nc.tensor.matmul(
                        st[:, lo:QC],
                        kt_t[:, 2 * c2:2 * c2 + 2, P * j:P * j + P],
                        qt_t[:, 2 * c2:2 * c2 + 2,
                             q8cols(QC * qc + lo):q8cols(QC * qc + QC)],
                        start=(c2 == 0), stop=(c2 == ND2 - 1),
                        perf_mode=DR)
                nc.scalar.activation(
                    pt[:, j, lo:QC], st[:, lo:QC],
                    mybir.ActivationFunctionType.Exp,
                    scale=SCALE, bias=bias_t[:])
                if r >= 0 and not (qc == 0 and j == 0):
                    # zero the dead q<k triangle of the diagonal block
                    nc.gpsimd.affine_select(
                        out=pt[:, j, lo:lo + P], in_=pt[:, j, lo:lo + P],
                        compare_op=mybir.AluOpType.is_ge,
                        fill=0.0, base=0,
                        pattern=[[1, P]], channel_multiplier=-1)

            for j in range(4 * qc + 4):
                units.append(lambda j=j: st_unit(j))
            if qc == 0:
                units.insert(1, hp_unit)
            return pt, units

        def stage_pv_units(b, qc, tiles, batch_state, pt, last=False):
            """Per-il emission thunks for PV + sums + recip + normalize + store."""
            kt_t, qt_t, v_t, kt0_t, qt0_t, v01_t = tiles
            s_bank = batch_state["s_bank"]
            ou = osb.tile([P, 4, D], u8, tag="ou")
            o_pair = {}

            def pv_unit(il):
                pt00 = batch_state.get("pt00")
                i = 4 * qc + il
                cols = slice(P * il, P * il + P)
                o_ps = pvp.tile([P, D], f32, tag="o")
                s_sl = s_bank[:, 8 * qc + 2 * il: 8 * qc + 2 * il + 2]
                if i == 0:
                    nc.tensor.matmul(o_ps[:], pt00[:], v01_t[:, 0, :],
                                     start=True, stop=True)
                    nc.tensor.matmul(s_sl, pt00[:], ones16[:],
                                     start=True, stop=True)
                elif i == 1:
                    # fp16 V blocks 0-1 (weights fp8): plain matmuls
                    for a in range(2):
                        nc.tensor.matmul(o_ps[:], pt[:, a, cols],
                                         v01_t[:, a, :],
                                         start=(a == 0), stop=(a == 1))
                    nc.tensor.matmul(s_sl, pt[:, 0:2, cols], ones8[:],
                                     start=True, stop=True, perf_mode=DR)
                else:
                    npair = (i + 1) // 2
                    leftover = (i + 1) - 2 * npair
                    for a in range(npair):
                        nc.tensor.matmul(
                            o_ps[:], pt[:, 2 * a:2 * a + 2, cols],
                            v_t[:, 2 * a:2 * a + 2, :],
                            start=(a == 0),
                            stop=(a == npair - 1 and not leftover),
                            perf_mode=DR)
                    if leftover:
                        nc.tensor.matmul(o_ps[:], pt[:, i, cols],
                                         v_t[:, i, :],
                                         start=False, stop=True)
                    for a in range(npair):
                        nc.tensor.matmul(
                            s_sl, pt[:, 2 * a:2 * a + 2, cols],
                            ones8[:],
                            start=(a == 0),
                            stop=(a == npair - 1 and not leftover),
                            perf_mode=DR)
                    if leftover:
                        nc.tensor.matmul(s_sl, pt[:, i, cols],
                                         ones8[:, 0, :],
                                         start=False, stop=True)
                o_pair[il] = o_ps
                if il % 2 == 1:
                    recip = sml.tile([P, 4], f32, tag="recip")
                    nc.vector.reciprocal(
                        recip[:],
                        s_bank[:, 8 * qc + 2 * il - 2: 8 * qc + 2 * il + 2])
                    for il2 in (il - 1, il):
                        r_sl = recip[:, 2 * (il2 % 2):2 * (il2 % 2) + 1]
                        if last and il2 % 2 == 1:
                            # tail only: ACT is idle; halve the DVE normalize
                            # chain. (If the ACT u8 cast truncates rather than
                            # rounds, these rows shift -half a quantum, which
                            # stays well inside the error budget.)
                            nc.scalar.activation(
                                ou[:, il2, :], o_pair[il2][:],
                                mybir.ActivationFunctionType.Copy,
                                scale=r_sl, bias=127.0)
                        else:
                            nc.vector.tensor_scalar(
                                out=ou[:, il2, :], in0=o_pair[il2][:],
                                scalar1=r_sl, scalar2=127.0,
                                op0=mybir.AluOpType.mult,
                                op1=mybir.AluOpType.add)
                    if last:
                        # final chunk: store per il-pair to shorten the tail
                        nc.sync.dma_start(
                            out=outd.ap()[b, qc][:, il - 1:il + 1],
                            in_=ou[:, il - 1:il + 1])
                    o_pair.clear()

            units = [lambda il=il: pv_unit(il) for il in range(4)]
            if not last:
                units.append(lambda: nc.sync.dma_start(
                    out=outd.ap()[b, qc], in_=ou[:]))
            return units

        ILK = stp_n

        def interleave(st_units, pv_units):
            """Emit the first ILK st units (fills the stp pool with no false
            fences on pv matmuls), then the whole pending pv stage, then the
            remaining st units. Engine sync is count-based, so consumers
            inherit every producer-engine instruction emitted before their
            producer -- order to keep those fences minimal."""
            st_units = list(st_units)
            for u in st_units[:ILK]:
                u()
            for u in pv_units:
                u()
            for u in st_units[ILK:]:
                u()

        for _ in range(repeat):
            nb = DBG_NB
            tiles = {0: emit_loads(0, fine=FINE)}
            if nb > 1:
                tiles[1] = emit_loads(1)
            states = {}
            pending = None  # args for the deferred pv stage
            for b in range(nb):
                states[b] = {"s_bank": smp.tile([P, 16], f32, tag="sbank",
                                                name=f"sbank{b}")}
                for qc in range(NQC):
                    pt, st_units = stage_st_units(b, qc, tiles[b], states[b])
                    pv_units = stage_pv_units(*pending) if pending else []
                    interleave(st_units, pv_units)
                    pending = (b, qc, tiles[b], states[b], pt)
                    if qc == 1 and b + 2 < nb:
                        tiles[b + 2] = emit_loads(b + 2)
            if pending is not None:
                for u in stage_pv_units(*pending, last=True):
                    u()
    nc.compile()
    return nc


def _get_nc(repeat: int = 1):
    key = repeat
    if key not in _NC_CACHE:
        _NC_CACHE[key] = _build(repeat)
    return _NC_CACHE[key]


def _pack_inputs(queries, keys, values):
    """Full tensors -> packed per-core DMA-friendly layouts."""
    f8 = ml_dtypes.float8_e4m3fn
    qf = np.asarray(queries, np.float32)
    kf = np.asarray(keys, np.float32)
    vf = np.asarray(values, np.float32)

    def pack_t(x, dt):  # [B, L, D] -> [B, 128, 4, L]
        xt = x.astype(dt).transpose(0, 2, 1).reshape(B, 4, P, LK)
        return np.ascontiguousarray(xt.transpose(0, 2, 1, 3))

    q8 = pack_t(qf, f8)[:, :, :, P:]            # drop dead first 128 q-cols
    k8 = pack_t(kf, f8)
    v8 = np.ascontiguousarray(
        vf.astype(f8).reshape(B, NJ, P, D).transpose(0, 2, 1, 3))
    kt0 = np.ascontiguousarray(
        kf[:, 0:P, :].astype(np.float16).transpose(0, 2, 1)
        .reshape(B, 4, P, P).transpose(0, 2, 1, 3))
    qt0 = np.ascontiguousarray(
        qf[:, 0:P, :].astype(np.float16).transpose(0, 2, 1)
        .reshape(B, 4, P, P).transpose(0, 2, 1, 3))
    v01 = np.ascontiguousarray(
        vf[:, 0:2 * P, :].astype(np.float16).reshape(B, 2, P, D)
        .transpose(0, 2, 1, 3))
    return q8, k8, v8, kt0, qt0, v01


def _unpack_out(out_u8):
    """[B, qc, p, il, d] u8 -> [B, LQ, D] f32; q = qc*512 + il*128 + p."""
    o = (out_u8.astype(np.float32) - 127.0) * OUT_SCALE
    return np.ascontiguousarray(
        o.transpose(0, 1, 3, 2, 4).reshape(B, LQ, D))


def _shard_inputs(queries, keys, values):
    q8, k8, v8, kt0, qt0, v01 = _pack_inputs(queries, keys, values)
    in_maps = []
    for c in range(N_CORES):
        s = slice(c * BPC, (c + 1) * BPC)
        in_maps.append({"qt": q8[s], "kt": k8[s], "v": v8[s],
                        "kt0": kt0[s], "qt0": qt0[s], "v01": v01[s]})
    return in_maps


def kernel(queries, keys, values, q_padding_mask=None, k_padding_mask=None,
           attn_mask=None, **_ignored):
    """Full-input entry point: shards batch over 8 NeuronCores, returns full output.

    The mask structure (no padding, causal attn_mask) is baked into the device
    kernel -- see module docstring.
    """
    nc = _get_nc()
    in_maps = _shard_inputs(queries, keys, values)
    res = run_bass_kernel_spmd(nc, in_maps, list(range(N_CORES)))
    out_p = np.concatenate([res.results[c]["out"] for c in range(N_CORES)], axis=0)
    return _unpack_out(out_p)
